# revision 11
# baseline (speedup 1.0000x reference)
"""Trainium2 Bass kernel for a 2-layer RGCN (basis decomposition) + readout.

Strategy (dst-node sharding, 8 cores):
  - Nodes are sharded contiguously across the 8 cores (3750 nodes/core).
  - Edges are routed to the core that owns their dst node and laid out in a
    per-core stream ordered by (slab of 256 dst nodes, relation). Each
    (slab, relation) bucket occupies a core-uniform [offset, offset+cap)
    range of the stream (cap = max edge count over cores), so one SPMD
    program serves all cores; per-core shortfall is padded with masked
    edges (dstloc=300 -> one-hot row is all zeros).
  - Per 128-edge chunk: dma_gather 128 source rows (bf16, 256B) from a
    replicated node table in HBM; DVE builds the scaled one-hot
    P[e, dstslot] = (iota==dstloc)*recip in one tensor_scalar; PE
    accumulates psum[feat, dstslot] += msgs.T @ P (fused segment-mean).
    Chunks on bucket boundaries are visited once per bucket with
    bucket-specific dstloc masks.
  - Per bucket: ACT copies psum -> SBUF bf16. Contraction
    out[h, n] = sum_r W_r.T @ agg_r + root.T @ own feats, ACT bias(+ReLU).
  - Between layers: PE transposes h1 to row-major, DMA to DRAM, AllGather
    rebuilds the full node table on every core.
  - Readout: out2.T (with a ones row) @ [wc; bc] per 128-node tile.
"""

import numpy as np
import ml_dtypes

bf16 = ml_dtypes.bfloat16

# ---------------------------------------------------------------- geometry

N, R, E = 30000, 16, 960000
IN, HID, OUT, ADR = 128, 128, 64, 400
M = 8                       # cores
NPC = N // M                # 3750
SLAB = 256
NSLAB = -(-NPC // SLAB)     # 15
NPC_PAD = NSLAB * SLAB      # 3840
CHUNK = 128
MT = M * NPC_PAD            # gather-table rows (30720)


class Prep:
    """Host-side preprocessing results."""
    pass


def host_preprocess(inputs):
    p = Prep()
    src = np.asarray(inputs["edge_index"][0]).astype(np.int64)
    dst = np.asarray(inputs["edge_index"][1]).astype(np.int64)
    et = np.asarray(inputs["edge_type"]).astype(np.int64)

    core = dst // NPC
    local = dst % NPC
    dstloc = (local % SLAB).astype(np.float32)

    seg = dst * R + et
    cnt = np.bincount(seg, minlength=N * R)
    recip = (1.0 / np.maximum(cnt[seg], 1.0)).astype(np.float32)

    src2 = (src // NPC) * NPC_PAD + (src % NPC)
    assert src2.max() < 32768

    slab = local // SLAB
    bucket = (core * NSLAB + slab) * R + et
    order = np.argsort(bucket, kind="stable")

    nb = M * NSLAB * R
    bcnt = np.bincount(bucket[order], minlength=nb).reshape(M, NSLAB, R)
    cap = bcnt.max(axis=0)                                   # [NSLAB, R]

    # bucket offsets within each slab's stream; slab streams padded to 128
    boff = np.zeros((NSLAB, R), np.int64)
    slab_len = np.zeros(NSLAB, np.int64)                     # padded slab lengths
    slab_off = np.zeros(NSLAB + 1, np.int64)
    for s in range(NSLAB):
        o = 0
        for r in range(R):
            boff[s, r] = o
            o += cap[s, r]
        slab_len[s] = -(-o // CHUNK) * CHUNK
        slab_off[s + 1] = slab_off[s] + slab_len[s]
    EPAD = int(slab_off[NSLAB])
    p.EPAD = EPAD
    p.cap, p.boff, p.slab_len, p.slab_off = cap, boff, slab_len, slab_off

    # chunk-uses: per (slab, r) the chunk range covering [boff, boff+cap)
    # use j -> (slab-local chunk index, column in dstloc/recip arrays)
    use_chunk = []          # per (s, r): list of slab-local chunk indices
    NUSE = 0
    for s in range(NSLAB):
        per_r = []
        nch_s = int(slab_len[s]) // CHUNK
        for s_r in range(R):
            r = s_r
            a = min(int(boff[s, r]) // CHUNK, nch_s - 1)
            b = -(-int(boff[s, r] + cap[s, r]) // CHUNK) if cap[s, r] > 0 else a + 1
            ch = list(range(a, min(max(b, a + 1), nch_s)))
            if not ch:
                ch = [nch_s - 1]
            per_r.append(ch)
            NUSE += len(ch)
        use_chunk.append(per_r)
    p.use_chunk = use_chunk
    p.NUSE = NUSE

    # per-core streams
    idx_a = np.zeros((M, EPAD), np.int16)
    dstloc_u = np.full((M, NUSE * CHUNK), 300.0, np.float32)
    recip_u = np.zeros((M, NUSE * CHUNK), np.float32)

    src2_s = src2[order].astype(np.int16)
    dstloc_s = dstloc[order]
    recip_s = recip[order]
    run_start = np.concatenate([[0], np.cumsum(bcnt.reshape(-1))[:-1]]).reshape(M, NSLAB, R)

    for c in range(M):
        for s in range(NSLAB):
            for r in range(R):
                n = int(bcnt[c, s, r])
                o = int(run_start[c, s, r])
                po = int(slab_off[s] + boff[s, r])
                idx_a[c, po:po + n] = src2_s[o:o + n]

    ucol = 0
    p.use_cols = []         # per (s, r): list of use-column indices
    for s in range(NSLAB):
        cols_r = []
        for r in range(R):
            cols = []
            for ch in use_chunk[s][r]:
                lo = ch * CHUNK                       # slab-local stream pos
                hi = lo + CHUNK
                b0, b1 = int(boff[s, r]), int(boff[s, r] + cap[s, r])
                for c in range(M):
                    n = int(bcnt[c, s, r])
                    o = int(run_start[c, s, r])
                    # stream positions [b0, b0+n) hold this core's edges
                    lo2, hi2 = max(lo, b0), min(hi, b0 + n)
                    if hi2 > lo2:
                        col = ucol * CHUNK
                        dstloc_u[c, col + (lo2 - lo):col + (hi2 - lo)] = \
                            dstloc_s[o + (lo2 - b0):o + (hi2 - b0)]
                        recip_u[c, col + (lo2 - lo):col + (hi2 - lo)] = \
                            recip_s[o + (lo2 - b0):o + (hi2 - b0)]
                cols.append(ucol)
                ucol += 1
            cols_r.append(cols)
        p.use_cols.append(cols_r)
    assert ucol == NUSE

    # idx wrapped [16, EPAD/16] then replicated 8x over partition groups
    p.idx = np.tile(idx_a.reshape(M, EPAD // 16, 16).transpose(0, 2, 1), (1, 8, 1))
    # dstloc/recip as [128, NUSE]
    p.dstloc = dstloc_u.reshape(M, NUSE, CHUNK).transpose(0, 2, 1).copy()
    p.recip = recip_u.reshape(M, NUSE, CHUNK).transpose(0, 2, 1).copy()

    comp1 = np.asarray(inputs["comp1"], np.float32)
    basis1 = np.asarray(inputs["basis1"], np.float32)
    comp2 = np.asarray(inputs["comp2"], np.float32)
    basis2 = np.asarray(inputs["basis2"], np.float32)
    p.W1 = np.einsum("rb,bio->rio", comp1, basis1).astype(bf16)
    p.W2 = np.einsum("rb,bio->rio", comp2, basis2).astype(bf16)
    p.root1 = np.asarray(inputs["root1"], np.float32).astype(bf16)
    p.root2 = np.asarray(inputs["root2"], np.float32).astype(bf16)
    p.bias1 = np.asarray(inputs["bias1"], np.float32).reshape(HID, 1)
    p.bias2 = np.asarray(inputs["bias2"], np.float32).reshape(OUT, 1)

    x = np.asarray(inputs["x"], np.float32)
    x_remap = np.zeros((MT, IN), bf16)
    for c in range(M):
        x_remap[c * NPC_PAD:c * NPC_PAD + NPC] = x[c * NPC:(c + 1) * NPC].astype(bf16)
    p.x_remap = x_remap

    wc_ext = np.zeros((128, ADR), bf16)
    wc_ext[:OUT] = np.asarray(inputs["wc"], np.float32).astype(bf16)
    wc_ext[OUT] = np.asarray(inputs["bc"], np.float32).astype(bf16)
    p.wc_ext = wc_ext

    p.iota = np.tile(np.arange(SLAB).astype(bf16), (128, 1))

    p.geom_key = (
        p.EPAD, p.NUSE,
        tuple(slab_len.tolist()),
        tuple(tuple(tuple(x) for x in sr) for sr in use_chunk),
    )
    return p


# ---------------------------------------------------------------- program

def build_program(p):
    import concourse.bacc as bacc
    import concourse.tile as tile
    import concourse.mybir as mybir
    from concourse import masks

    dt = mybir.dt
    Alu = mybir.AluOpType
    Act = mybir.ActivationFunctionType

    EPAD, NUSE = p.EPAD, p.NUSE
    slab_len, slab_off = p.slab_len, p.slab_off
    use_chunk, use_cols = p.use_chunk, p.use_cols
    maxC = int(max(slab_len)) // CHUNK

    nc = bacc.Bacc("TRN2", target_bir_lowering=False, debug=False, num_devices=M)

    x_remap = nc.dram_tensor("x_remap", [MT, IN], dt.bfloat16, kind="ExternalInput")
    x_own = nc.dram_tensor("x_own", [NPC_PAD, IN], dt.bfloat16, kind="ExternalInput")
    idx_d = nc.dram_tensor("idx", [128, EPAD // 16], dt.int16, kind="ExternalInput")
    dstloc_d = nc.dram_tensor("dstloc", [128, NUSE], dt.float32, kind="ExternalInput")
    recip_d = nc.dram_tensor("recip", [128, NUSE], dt.float32, kind="ExternalInput")
    w1_d = nc.dram_tensor("w1", [R, IN, HID], dt.bfloat16, kind="ExternalInput")
    w2_d = nc.dram_tensor("w2", [R, HID, OUT], dt.bfloat16, kind="ExternalInput")
    root1_d = nc.dram_tensor("root1", [IN, HID], dt.bfloat16, kind="ExternalInput")
    root2_d = nc.dram_tensor("root2", [HID, OUT], dt.bfloat16, kind="ExternalInput")
    bias1_d = nc.dram_tensor("bias1", [HID, 1], dt.float32, kind="ExternalInput")
    bias2_d = nc.dram_tensor("bias2", [OUT, 1], dt.float32, kind="ExternalInput")
    wc_d = nc.dram_tensor("wc_ext", [128, ADR], dt.bfloat16, kind="ExternalInput")
    iota_d = nc.dram_tensor("iota", [128, SLAB], dt.bfloat16, kind="ExternalInput")
    out_d = nc.dram_tensor("out", [NPC, ADR], dt.float32, kind="ExternalOutput")
    h1_bounce = nc.dram_tensor("h1_bounce", [NPC_PAD, HID], dt.bfloat16)
    h1_table = nc.dram_tensor("h1_table", [MT, HID], dt.bfloat16)

    NT = NPC_PAD // 128

    with tile.TileContext(nc) as tc:
        with (
            tc.tile_pool(name="const", bufs=1) as cpool,
            tc.tile_pool(name="msgs", bufs=2) as gpool,
            tc.tile_pool(name="pmat", bufs=6) as ppool,
            tc.tile_pool(name="agg", bufs=R + 2) as apool,
            tc.tile_pool(name="osb", bufs=2) as opool,
            tc.tile_pool(name="psA", bufs=3, space="PSUM") as psA,
            tc.tile_pool(name="psB", bufs=2, space="PSUM") as psB,
            tc.tile_pool(name="psT", bufs=2, space="PSUM") as psT,
            tc.tile_pool(name="psF", bufs=1, space="PSUM") as psF,
        ):
            def cload(name, dram, shape, dtype):
                t = cpool.tile(shape, dtype, tag=name)
                nc.sync.dma_start(t[:], dram.ap())
                return t

            iota_sb = cload("iota", iota_d, [128, SLAB], dt.bfloat16)
            idx_sb = cload("idx", idx_d, [128, EPAD // 16], dt.int16)
            dstloc_sb = cload("dstloc", dstloc_d, [128, NUSE], dt.float32)
            recip_sb = cload("recip", recip_d, [128, NUSE], dt.float32)
            bias1_sb = cload("bias1", bias1_d, [HID, 1], dt.float32)
            bias2_sb = cload("bias2", bias2_d, [OUT, 1], dt.float32)
            wc_sb = cload("wc", wc_d, [128, ADR], dt.bfloat16)
            root1_sb = cload("root1", root1_d, [IN, HID], dt.bfloat16)
            root2_sb = cload("root2", root2_d, [HID, OUT], dt.bfloat16)

            w1_sb = cpool.tile([IN, R, HID], dt.bfloat16, tag="w1")
            nc.sync.dma_start(w1_sb[:], w1_d.ap().rearrange("r i h -> i r h"))
            w2_sb = cpool.tile([HID, R, OUT], dt.bfloat16, tag="w2")
            nc.sync.dma_start(w2_sb[:], w2_d.ap().rearrange("r i h -> i r h"))

            xT_sb = cpool.tile([128, NPC_PAD], dt.bfloat16, tag="xT")
            nc.sync.dma_start_transpose(xT_sb[:], x_own.ap())

            ident_sb = cpool.tile([128, 128], dt.bfloat16, tag="ident")
            masks.make_identity(nc, ident_sb[:])

            h1T_sb = cpool.tile([128, NPC_PAD], dt.bfloat16, tag="h1T")
            out2T_sb = cpool.tile([128, NPC_PAD], dt.bfloat16, tag="out2T")
            h1rows_sb = cpool.tile([128, NT, HID], dt.bfloat16, tag="h1rows")

            nc.vector.memset(out2T_sb[OUT:128, :], 0.0)
            nc.vector.memset(out2T_sb[OUT:OUT + 1, :], 1.0)

            def layer(table_ap, rootsrc_sb, w_sb, root_sb, bias_sb, DO, relu, hT_sb):
                gathers = []
                for s in range(NSLAB):
                    SG = int(slab_len[s])
                    off = int(slab_off[s])
                    msgs = gpool.tile([128, maxC, IN], dt.bfloat16, tag="msgs")
                    nc.gpsimd.dma_gather(
                        out_ap=msgs[:, :SG // CHUNK, :],
                        in_ap=table_ap,
                        idxs_ap=idx_sb[:, off // 16:(off + SG) // 16],
                        num_idxs=SG,
                        num_idxs_reg=SG,
                        elem_size=IN,
                        single_packet=False,
                    )
                    gathers.append(list(nc.all_instructions())[-1])
                    aggs = []
                    for r in range(R):
                        chunks = use_chunk[s][r]
                        cols = use_cols[s][r]
                        ps = psA.tile([128, SLAB], dt.float32, tag="psA")
                        for k, (ch, col) in enumerate(zip(chunks, cols)):
                            P = ppool.tile([128, SLAB], dt.bfloat16, tag="P")
                            nc.vector.tensor_scalar(
                                P[:], iota_sb[:],
                                dstloc_sb[:, col:col + 1], recip_sb[:, col:col + 1],
                                Alu.is_equal, Alu.mult,
                            )
                            nc.tensor.matmul(
                                ps[:], msgs[:, ch, :], P[:],
                                start=(k == 0), stop=(k == len(chunks) - 1),
                            )
                        a = apool.tile([128, SLAB], dt.bfloat16, tag="agg")
                        nc.scalar.activation(a[:], ps[:], Act.Copy)
                        aggs.append(a)
                    po = psB.tile([DO, SLAB], dt.float32, tag="psB")
                    for r in range(R):
                        nc.tensor.matmul(
                            po[:], w_sb[:, r, :], aggs[r][:],
                            start=(r == 0), stop=False,
                        )
                    nc.tensor.matmul(
                        po[:], root_sb[:],
                        rootsrc_sb[:, s * SLAB:(s + 1) * SLAB],
                        start=False, stop=True,
                    )
                    nc.scalar.activation(
                        hT_sb[:DO, s * SLAB:(s + 1) * SLAB], po[:],
                        Act.Relu if relu else Act.Identity,
                        bias=bias_sb[:],
                    )
                return gathers

            # ---- layer 1
            layer(x_remap.ap(), xT_sb, w1_sb, root1_sb, bias1_sb, HID, True, h1T_sb)

            # ---- h1 -> row-major -> DRAM -> AllGather
            for t in range(NT):
                pst = psT.tile([128, 128], dt.bfloat16, tag="psT")
                nc.tensor.matmul(
                    pst[:], h1T_sb[:, t * 128:(t + 1) * 128], ident_sb[:],
                    is_transpose=True,
                )
                nc.vector.tensor_copy(h1rows_sb[:, t, :], pst[:])
            nc.sync.dma_start(
                h1_bounce.ap().rearrange("(t p) f -> p t f", p=128),
                h1rows_sb[:],
            )
            nc.gpsimd.collective_compute(
                "AllGather",
                mybir.AluOpType.bypass,
                replica_groups=[list(range(M))],
                ins=[h1_bounce.ap().opt()],
                outs=[h1_table.ap().opt()],
            )
            coll = list(nc.all_instructions())[-1]

            # ---- layer 2
            gathers2 = layer(h1_table.ap(), h1T_sb, w2_sb, root2_sb, bias2_sb,
                             OUT, False, out2T_sb)
            for g in gathers2:
                tile.add_dep_helper(g, coll, reason="gather after allgather")

            # ---- readout
            for t in range(NT):
                psf = psF.tile([128, ADR], dt.float32, tag="psF")
                nc.tensor.matmul(
                    psf[:], out2T_sb[:, t * 128:(t + 1) * 128], wc_sb[:],
                    start=True, stop=True,
                )
                ot = opool.tile([128, ADR], dt.float32, tag="osb")
                nc.vector.tensor_copy(ot[:], psf[:])
                rows = min(128, NPC - t * 128)
                if rows > 0:
                    nc.sync.dma_start(out_d[t * 128:t * 128 + rows, :], ot[:rows, :])

    nc.compile()
    return nc


# ---------------------------------------------------------------- runner

_CACHE = {}


def make_in_maps(p):
    shared = dict(
        x_remap=p.x_remap,
        w1=np.ascontiguousarray(p.W1), w2=np.ascontiguousarray(p.W2),
        root1=p.root1, root2=p.root2,
        bias1=p.bias1, bias2=p.bias2,
        wc_ext=p.wc_ext, iota=p.iota,
    )
    in_maps = []
    for c in range(M):
        m = dict(shared)
        m["x_own"] = np.ascontiguousarray(p.x_remap[c * NPC_PAD:(c + 1) * NPC_PAD])
        m["idx"] = np.ascontiguousarray(p.idx[c])
        m["dstloc"] = np.ascontiguousarray(p.dstloc[c])
        m["recip"] = np.ascontiguousarray(p.recip[c])
        in_maps.append(m)
    return in_maps


def get_program(p):
    key = p.geom_key
    if key not in _CACHE:
        _CACHE[key] = build_program(p)
    return _CACHE[key]


def run(p, trace=False, **kw):
    from concourse.bass_utils import run_bass_kernel_spmd
    nc = get_program(p)
    res = run_bass_kernel_spmd(nc, make_in_maps(p), core_ids=list(range(M)),
                               trace=trace, **kw)
    out = np.concatenate([res.results[c]["out"] for c in range(M)], axis=0)
    return out.astype(np.float32), res


def kernel(**inputs):
    p = host_preprocess(inputs)
    out, _ = run(p)
    return out
